# revision 1
# baseline (speedup 1.0000x reference)
"""PerceptualMelLoss on 8 trn2 NeuronCores.

Data-parallel over batch (8 items/core). Each core computes masked partial
sums for all six loss reductions; host finishes the weighted formula in f64.

Device layout per item: (4000, 80) frames viewed as [125 partitions, 32, 80]
(frame f = 32*p + j). Masked reductions run on the PE: stationary = mask
column [125, 1] bf16, moving = |quantity| [125, N] bf16, accumulated into
PSUM f32 across all j and all 8 items.
"""

import numpy as np

import bass_rust as _bass_rust
import concourse.bass as bass
import concourse.tile as tile
from concourse.bass_utils import run_bass_kernel_spmd
from concourse import mybir

NCORES = 8
B, T, D = 64, 4000, 80
BPC = B // NCORES          # items per core
P, J = 125, 32             # T = P*J

F32 = mybir.dt.float32
BF16 = mybir.dt.bfloat16
ALU = mybir.AluOpType
AF = mybir.ActivationFunctionType
AX = mybir.AxisListType

W_L1, W_DELTA, W_DELTA2, W_SC, W_BAND, W_ENERGY = 1.0, 0.5, 0.25, 0.5, 1.0, 0.5
EPS = 1e-8

_NC = None


def _ptile(tc, shape, dtype, name, **kw):
    t, _free = tc.tile(shape, dtype, name=name, **kw)
    return t


def _build_nc():
    nc = bass.Bass()
    pred = nc.dram_tensor("pred", [BPC, T, D], F32, kind="ExternalInput")
    targ = nc.dram_tensor("targ", [BPC, T, D], F32, kind="ExternalInput")
    mask = nc.dram_tensor("mask", [BPC, T], F32, kind="ExternalInput")
    sums = nc.dram_tensor("sums", [1, 400], F32, kind="ExternalOutput")
    se_out = nc.dram_tensor("se", [P, BPC], F32, kind="ExternalOutput")

    with tile.TileContext(nc) as tc, \
         tc.tile_pool(name="persist", bufs=1) as ppool, \
         tc.tile_pool(name="psum", bufs=1,
                      space=bass.MemorySpace.PSUM) as psum_pool:
        # ---------- persistent tiles ----------
        m_f32 = ppool.tile([P, BPC, J], F32, name="m_f32")
        m_bf = ppool.tile([P, BPC, J], BF16, name="m_bf")
        mshift_f = ppool.tile([P, BPC], F32, name="mshift_f")
        mdown_f = ppool.tile([P, BPC], F32, name="mdown_f")
        mshift_bf = ppool.tile([P, BPC], BF16, name="mshift_bf")
        mdown_bf = ppool.tile([P, BPC], BF16, name="mdown_bf")
        dm = ppool.tile([P, BPC, J - 1], BF16, name="dm")
        dm2 = ppool.tile([P, BPC, J - 2], BF16, name="dm2")
        dmx = ppool.tile([P, BPC], BF16, name="dmx")
        dm30 = ppool.tile([P, BPC], BF16, name="dm30")
        dm31 = ppool.tile([P, BPC], BF16, name="dm31")
        acc_se = ppool.tile([P, BPC], F32, name="acc_se")
        staging = ppool.tile([1, 400], F32, name="staging")

        psum_aqq = psum_pool.tile([1, 3, D], F32, name="psum_aqq")
        psum_d = psum_pool.tile([1, D], F32, name="psum_d")
        psum_d2 = psum_pool.tile([1, D], F32, name="psum_d2")

        # ---------- mask preprocessing (once per core) ----------
        nc.sync.dma_start(out=m_f32[:],
                          in_=mask.rearrange("b (p j) -> p b j", p=P))
        nc.vector.memset(mshift_f[0:1, :], 0.0)
        nc.vector.memset(mdown_f[:], 0.0)
        # mshift[p, b] = mask[b, 32p-1] ; mdown[p, b] = mask[b, 32(p+1)]
        nc.sync.dma_start(out=mshift_f[1:P, :], in_=m_f32[0:P - 1, :, J - 1])
        nc.sync.dma_start(out=mdown_f[0:P - 1, :], in_=m_f32[1:P, :, 0])

        nc.vector.tensor_scalar(m_bf[:], m_f32[:], 0.0, None, op0=ALU.add)
        nc.vector.tensor_scalar(mshift_bf[:], mshift_f[:], 0.0, None, op0=ALU.add)
        nc.vector.tensor_scalar(mdown_bf[:], mdown_f[:], 0.0, None, op0=ALU.add)

        nc.vector.tensor_tensor(dm[:], m_bf[:, :, 0:J - 1], m_bf[:, :, 1:J],
                                op=ALU.mult)
        nc.vector.tensor_tensor(dm2[:], dm[:, :, 0:J - 2], m_bf[:, :, 2:J],
                                op=ALU.mult)
        nc.vector.tensor_tensor(dmx[:], mshift_bf[:], m_bf[:, :, 0], op=ALU.mult)
        nc.vector.tensor_tensor(dm30[:], dm[:, :, J - 2], mdown_bf[:], op=ALU.mult)
        nc.vector.tensor_tensor(dm31[:], dmx[:], m_bf[:, :, 1], op=ALU.mult)

        # ---------- per-item pipeline ----------
        with tc.tile_pool(name="work", bufs=2) as pool:
            for b in range(BPC):
                pv = pred[b].rearrange("(p j) d -> p j d", p=P)
                tv = targ[b].rearrange("(p j) d -> p j d", p=P)

                Pt = pool.tile([P, J, D], F32, name="Pt")
                Tt = pool.tile([P, J, D], F32, name="Tt")
                nc.sync.dma_start(out=Pt[:], in_=pv)
                nc.sync.dma_start(out=Tt[:], in_=tv)

                # boundary rows straight from DRAM
                Psh = pool.tile([P, D], F32, name="Psh")
                Tsh = pool.tile([P, D], F32, name="Tsh")
                Pdn = pool.tile([P, D], F32, name="Pdn")
                Tdn = pool.tile([P, D], F32, name="Tdn")
                nc.sync.dma_start(out=Psh[1:P, :], in_=pv[0:P - 1, J - 1, :])
                nc.sync.dma_start(out=Tsh[1:P, :], in_=tv[0:P - 1, J - 1, :])
                nc.sync.dma_start(out=Pdn[0:P - 1, :], in_=pv[1:P, 0, :])
                nc.sync.dma_start(out=Tdn[0:P - 1, :], in_=tv[1:P, 0, :])

                E = pool.tile([P, J, D], BF16, name="E")
                nc.vector.tensor_tensor(E[:], Pt[:], Tt[:], op=ALU.subtract)

                # A = |E|, Q = E^2, Qt = T^2, j-major for contiguous moving APs
                AQQ = pool.tile([P, J, 3, D], BF16, name="AQQ")
                nc.scalar.activation(AQQ[:, :, 0, :], E[:], AF.Abs)
                nc.scalar.activation(AQQ[:, :, 1, :], E[:], AF.Square)
                nc.scalar.activation(AQQ[:, :, 2, :], Tt[:], AF.Square)

                D1 = pool.tile([P, J - 1, D], BF16, name="D1")
                nc.vector.tensor_tensor(D1[:], E[:, 1:J, :], E[:, 0:J - 1, :],
                                        op=ALU.subtract)
                D2t = pool.tile([P, J - 2, D], BF16, name="D2t")
                nc.vector.tensor_tensor(D2t[:], D1[:, 1:J - 1, :],
                                        D1[:, 0:J - 2, :], op=ALU.subtract)
                A1 = pool.tile([P, J - 1, D], BF16, name="A1")
                nc.scalar.activation(A1[:], D1[:], AF.Abs)
                A2 = pool.tile([P, J - 2, D], BF16, name="A2")
                nc.scalar.activation(A2[:], D2t[:], AF.Abs)

                # cross-partition boundary deltas
                Eshift = pool.tile([P, D], BF16, name="Eshift")
                Edown = pool.tile([P, D], BF16, name="Edown")
                nc.vector.tensor_tensor(Eshift[:], Psh[:], Tsh[:], op=ALU.subtract)
                nc.vector.memset(Eshift[0:1, :], 0.0)
                nc.vector.memset(Edown[:], 0.0)
                nc.vector.tensor_tensor(Edown[0:P - 1, :], Pdn[0:P - 1, :],
                                        Tdn[0:P - 1, :], op=ALU.subtract)

                Dx = pool.tile([P, D], BF16, name="Dx")
                nc.vector.tensor_tensor(Dx[:], E[:, 0, :], Eshift[:],
                                        op=ALU.subtract)
                Ddown = pool.tile([P, D], BF16, name="Ddown")
                nc.vector.tensor_tensor(Ddown[:], Edown[:], E[:, J - 1, :],
                                        op=ALU.subtract)
                D2j30 = pool.tile([P, D], BF16, name="D2j30")
                nc.vector.tensor_tensor(D2j30[:], Ddown[:], D1[:, J - 2, :],
                                        op=ALU.subtract)
                D2last = pool.tile([P, D], BF16, name="D2last")
                nc.vector.tensor_tensor(D2last[:], D1[:, 0, :], Dx[:],
                                        op=ALU.subtract)

                ADx = pool.tile([P, D], BF16, name="ADx")
                A2j30 = pool.tile([P, D], BF16, name="A2j30")
                A2last = pool.tile([P, D], BF16, name="A2last")
                nc.scalar.activation(ADx[:], Dx[:], AF.Abs)
                nc.scalar.activation(A2j30[:], D2j30[:], AF.Abs)
                nc.scalar.activation(A2last[:], D2last[:], AF.Abs)

                # energy: R = sum_d E per frame, then sum_f m*|R| per partition
                R = pool.tile([P, J], BF16, name="R")
                with nc.allow_low_precision("bf16 R validated: ~5e-6 rel err"):
                    nc.vector.tensor_reduce(R[:], E[:], axis=AX.X, op=ALU.add)
                RM = pool.tile([P, J], BF16, name="RM")
                nc.vector.tensor_tensor(RM[:], R[:], m_bf[:, b, :], op=ALU.mult)
                nc.vector.tensor_reduce(acc_se[:, b:b + 1], RM[:], axis=AX.X,
                                        op=ALU.add, apply_absolute_value=True)

                # PE masked reductions
                for j in range(J):
                    nc.tensor.matmul(psum_aqq[:], m_bf[:, b, j:j + 1],
                                     AQQ[:, j], start=(b == 0 and j == 0),
                                     stop=(b == BPC - 1 and j == J - 1))
                for j in range(J - 1):
                    nc.tensor.matmul(psum_d[:], dm[:, b, j:j + 1], A1[:, j, :],
                                     start=(b == 0 and j == 0), stop=False)
                nc.tensor.matmul(psum_d[:], dmx[:, b:b + 1], ADx[:],
                                 start=False, stop=(b == BPC - 1))
                for j in range(J - 2):
                    nc.tensor.matmul(psum_d2[:], dm2[:, b, j:j + 1], A2[:, j, :],
                                     start=(b == 0 and j == 0), stop=False)
                nc.tensor.matmul(psum_d2[:], dm30[:, b:b + 1], A2j30[:],
                                 start=False, stop=False)
                nc.tensor.matmul(psum_d2[:], dm31[:, b:b + 1], A2last[:],
                                 start=False, stop=(b == BPC - 1))

        # ---------- drain results ----------
        nc.vector.tensor_scalar(staging[:, 0:240].rearrange("a (b c) -> a b c", b=3),
                                psum_aqq[:], 0.0, None, op0=ALU.add)
        nc.vector.tensor_scalar(staging[:, 240:320], psum_d[:], 0.0, None,
                                op0=ALU.add)
        nc.vector.tensor_scalar(staging[:, 320:400], psum_d2[:], 0.0, None,
                                op0=ALU.add)
        nc.sync.dma_start(out=sums[:], in_=staging[:])
        nc.sync.dma_start(out=se_out[:], in_=acc_se[:])

    # TRN2 allows at most one semaphore wait per instruction; this pass
    # splits multi-wait instructions via InstEventSemaphore.
    _bass_rust.generate_event_semaphores(nc)
    return nc


def kernel(pred_mel, target_mel, mel_mask, band_weights):
    global _NC
    if _NC is None:
        _NC = _build_nc()

    pred_mel = np.ascontiguousarray(pred_mel, dtype=np.float32)
    target_mel = np.ascontiguousarray(target_mel, dtype=np.float32)
    mel_mask = np.ascontiguousarray(mel_mask, dtype=np.float32)

    in_maps = []
    for c in range(NCORES):
        s = slice(c * BPC, (c + 1) * BPC)
        in_maps.append({
            "pred": pred_mel[s],
            "targ": target_mel[s],
            "mask": mel_mask[s],
        })

    res = run_bass_kernel_spmd(_NC, in_maps, list(range(NCORES)))

    sums = np.zeros(400, dtype=np.float64)
    se = 0.0
    for r in res.results:
        sums += r["sums"].reshape(400).astype(np.float64)
        se += float(r["se"].astype(np.float64).sum())

    s1d = sums[0:80]
    s_num = sums[80:160].sum()
    s_den = sums[160:240].sum()
    s_d = sums[240:320].sum()
    s_d2 = sums[320:400].sum()
    s1 = s1d.sum()

    m = mel_mask.astype(np.float64)
    cm = m.sum()
    cd = (m[:, 1:] * m[:, :-1]).sum()
    cd2 = (m[:, 2:] * m[:, 1:-1] * m[:, :-2]).sum()

    n1 = max(D * cm, 1.0)
    l1 = s1 / n1
    delta = s_d / max(D * cd, 1.0)
    delta2 = s_d2 / max(D * cd2, 1.0)
    sc = np.sqrt(s_num / n1) / max(np.sqrt(s_den / n1), EPS)
    w = band_weights.astype(np.float64)
    band = (s1d @ w) / n1 / w.mean()
    energy = (se / D) / max(cm, 1.0)

    total = (W_L1 * l1 + W_DELTA * delta + W_DELTA2 * delta2
             + W_SC * sc + W_BAND * band + W_ENERGY * energy)
    return np.float32(total)

